# revision 11
# baseline (speedup 1.0000x reference)
"""3-layer GAT on 8 Trainium2 NeuronCores.

Strategy: destination-node 1D partition. Core c owns dst nodes
[c*NV, (c+1)*NV) and every edge pointing at them (sorted by dst).
Per layer:
  Phase M: each core computes its row-slice of h_aug = x @ [W | W a_src | W a_dst],
           packs a bf16 gather table row-slice (h bf16, asrc hi/lo bf16, ones col),
           AllGather -> full table in DRAM.
  Phase G: per 128-dst block: dma_gather the block's edge source rows
           (edge-major), tiny M01^T matmuls broadcast a_dst to edges,
           p = exp(leakyrelu(asrc+adst) - 8) on ACT, S = M01 * p on DVE,
           128-edge-chunk matmuls (lhsT=S) accumulate  sum_e p_e * h_aug[src_e]
           into PSUM; trailing ones-column accumulates the softmax denominator.
           Normalize *after* aggregation (coeff = p/denom cancels), fused relu.
Segment-max of the softmax is replaced by a constant shift (exp can't
overflow fp32 for this data; the shift cancels exactly in the normalize).
int16 gather indices can't span 50000 rows -> per block the edges are
split into A (src < 32768, table base 0) and B (src >= 32768, base-offset
view), each padded to 128-edge chunks.
"""

import sys, os
sys.path.insert(0, "/opt/trn_rl_repo")

import numpy as np
import ml_dtypes

BF16 = ml_dtypes.bfloat16

# problem dims (hardcoded per task spec)
N_NODES = 50000
N_EDGES = 800000
D_IN = 256
D_H = 256
N_CLS = 10
NCORES = 8
NV = N_NODES // NCORES          # 6250 dst nodes per core
SPLIT = 32768                   # int16 gather index split
CEXP = 8.0                      # global shift inside exp (cancels in softmax)
NEG_SLOPE = 0.2
TW12 = 384                      # bf16 table row width, layers 1/2 (768B)
TW3 = 128                       # layer 3 (256B)
MAX_W = 128                     # max chunk dst-window width (after 32-align)


# ----------------------------------------------------------------------------
# host preprocessing
# ----------------------------------------------------------------------------

def _wrap_idx(idx):
    """[n] int16 -> [128, n/16] wrapped in 16 partitions, replicated 8x."""
    n = len(idx)
    assert n % 16 == 0
    w = np.asarray(idx, np.int16).reshape(n // 16, 16).T.copy()  # [16, n/16]
    return np.tile(w, (8, 1))


def preprocess(edge_index, n_nodes=N_NODES, ncores=NCORES, nv=NV, split=SPLIT):
    """Partition + chunk the graph. Returns (meta, per_core) where meta is
    core-independent program structure and per_core holds idx/M01 blobs."""
    src_all = np.concatenate([edge_index[0], np.arange(n_nodes, dtype=np.int64)])
    dst_all = np.concatenate([edge_index[1], np.arange(n_nodes, dtype=np.int64)])

    nblk = (nv + 127) // 128
    blkw = [min(128, nv - b * 128) for b in range(nblk)]

    # per (core, block, half): sorted (src, dst_local_in_block) lists
    per = [[[None, None] for _ in range(nblk)] for _ in range(ncores)]
    for c in range(ncores):
        m = (dst_all >= c * nv) & (dst_all < (c + 1) * nv)
        s, d = src_all[m], dst_all[m] - c * nv
        o = np.argsort(d, kind="stable")
        s, d = s[o], d[o]
        for b in range(nblk):
            mb = (d >= b * 128) & (d < b * 128 + blkw[b])
            sb_, db_ = s[mb], d[mb] - b * 128
            ma = sb_ < split
            per[c][b][0] = (sb_[ma], db_[ma])
            per[c][b][1] = (sb_[~ma] - split, db_[~ma])

    ncha = [0] * nblk
    nchb = [0] * nblk
    for b in range(nblk):
        ncha[b] = max((len(per[c][b][0][0]) + 127) // 128 for c in range(ncores))
        nchb[b] = max((len(per[c][b][1][0]) + 127) // 128 for c in range(ncores))

    # chunk windows (32-aligned base, union over cores)
    win = []           # win[b][j] = (base, w)
    for b in range(nblk):
        wl = []
        for j in range(ncha[b] + nchb[b]):
            half, jj = (0, j) if j < ncha[b] else (1, j - ncha[b])
            lo, hi = 128, -1
            for c in range(ncores):
                dl = per[c][b][half][1][jj * 128:(jj + 1) * 128]
                if len(dl):
                    lo, hi = min(lo, dl.min()), max(hi, dl.max())
            if hi < 0:          # no core has edges in this chunk (can't happen)
                lo, hi = 0, 0
            # HW rule: partition APs from base 32 must stay within [32, 64),
            # from base 64 within [64, 128); base 0 can span all 128.
            end = int(hi) + 1
            if lo >= 64:
                base = 64
            elif lo >= 32 and end <= 64:
                base = 32
            else:
                base = 0
            w = end - base
            assert w <= MAX_W, (b, j, base, hi)
            wl.append((int(base), int(w)))
        win.append(wl)

    # blob layouts
    idx_off = []       # slot offset of block's A group; B follows at +ncha*128
    tot_slots = 0
    for b in range(nblk):
        idx_off.append(tot_slots)
        tot_slots += (ncha[b] + nchb[b]) * 128

    # per-block mblob: [m01 chunk cols ...][m01t packed 128-col groups]
    # m01t packing: chunks with w<=64 pair up at partition rows {0, 64};
    # wider chunks take a full 128-col group alone at partition row 0.
    m01_off = []       # m01_off[b][j] = col offset of chunk j's M01 inside mblob b
    m01t_pos = []      # m01t_pos[b][j] = (partition_row, col offset inside mblob b)
    mblob_off = []     # mblob_off[b] = col offset of block b inside the big blob
    mblob_len = []
    tot_mcols = 0
    for b in range(nblk):
        nch = ncha[b] + nchb[b]
        offs, o = [], 0
        for j in range(nch):
            offs.append(o)
            o += win[b][j][1]
        m01_off.append(offs)
        # each chunk's M01^T lives at partition rows [base, base+w) (must
        # match the adst rhs slice base partition); greedily pack chunks
        # with disjoint partition ranges into shared 128-col groups.
        cur_col, occ, pos = o, None, []
        for j in range(nch):
            base, w = win[b][j]
            rng = (base, base + w)
            if occ is None:
                occ = [rng]
            elif all(rng[1] <= r0 or rng[0] >= r1 for (r0, r1) in occ):
                occ.append(rng)
            else:
                cur_col += 128
                occ = [rng]
            pos.append((base, cur_col))
        if occ is not None:
            cur_col += 128
        m01t_pos.append(pos)
        mblob_off.append(tot_mcols)
        mblob_len.append(cur_col)
        tot_mcols += cur_col

    meta = dict(nblk=nblk, blkw=blkw, ncha=ncha, nchb=nchb, win=win,
                idx_off=idx_off, tot_slots=tot_slots, m01_off=m01_off,
                m01t_pos=m01t_pos, mblob_off=mblob_off, mblob_len=mblob_len,
                tot_mcols=tot_mcols)

    # per-core blobs
    per_core = []
    for c in range(ncores):
        idx = np.zeros(tot_slots, np.int16)
        mblob = np.zeros((128, tot_mcols), BF16)
        srcs = np.zeros(tot_slots, np.int64)     # numpy-model convenience
        valid = np.zeros(tot_slots, bool)
        dloc = np.zeros(tot_slots, np.int64)
        for b in range(nblk):
            nch = ncha[b] + nchb[b]
            for j in range(nch):
                half, jj = (0, j) if j < ncha[b] else (1, j - ncha[b])
                s_, d_ = per[c][b][half]
                s_, d_ = s_[jj * 128:(jj + 1) * 128], d_[jj * 128:(jj + 1) * 128]
                n = len(s_)
                base, w = win[b][j]
                slot0 = idx_off[b] + j * 128
                if n:
                    idx[slot0:slot0 + n] = s_.astype(np.int16)
                    srcs[slot0:slot0 + n] = s_ + (split if half else 0)
                    valid[slot0:slot0 + n] = True
                    dloc[slot0:slot0 + n] = d_
                    col = mblob_off[b] + m01_off[b][j]
                    e = np.arange(n)
                    m01 = np.zeros((128, w), BF16)
                    m01[e, d_ - base] = 1
                    mblob[:, col:col + w] = m01
                    prow, tc_rel = m01t_pos[b][j]
                    tcol = mblob_off[b] + tc_rel
                    m01t = np.zeros((w, 128), BF16)
                    m01t[d_ - base, e] = 1
                    mblob[prow:prow + w, tcol:tcol + 128] = m01t
        # wrapped idx per gather group (per block-half), concatenated
        wrapped = []
        for b in range(nblk):
            sA = ncha[b] * 128
            sB = nchb[b] * 128
            o = idx_off[b]
            if sA:
                wrapped.append(_wrap_idx(idx[o:o + sA]))
            if sB:
                wrapped.append(_wrap_idx(idx[o + sA:o + sA + sB]))
        idx_wrapped = np.concatenate(wrapped, axis=1)
        per_core.append(dict(idx=idx_wrapped, mblob=mblob, srcs=srcs,
                             valid=valid, dloc=dloc))
    return meta, per_core


def make_waug(W, a_src, a_dst):
    return np.concatenate(
        [W, (W @ a_src)[:, None], (W @ a_dst)[:, None]], axis=1
    ).astype(np.float32)


# ----------------------------------------------------------------------------
# numpy model of the exact kernel math (for numerics validation)
# ----------------------------------------------------------------------------

def numpy_model(x, meta, per_core, waugs, nv=NV, ncores=NCORES):
    f32 = np.float32
    bf = lambda a: a.astype(BF16)
    outs = []
    xs = x.astype(f32)
    dims = [(D_H, TW12), (D_H, TW12), (N_CLS, TW3)]
    act = xs
    for li, (wa, (dh, tw)) in enumerate(zip(waugs, dims)):
        # phase M (all cores' slices -> full table)
        h = act @ wa                       # [N, dh+2] fp32
        asrc = h[:, dh].astype(f32)
        adst = h[:, dh + 1].astype(f32)
        tab = np.zeros((act.shape[0], tw), BF16)
        tab[:, :dh] = bf(h[:, :dh])
        hi = bf(asrc)
        lo = bf(asrc - hi.astype(f32))
        tab[:, dh] = hi
        tab[:, dh + 1] = lo
        tab[:, dh + 2] = BF16(1.0)
        # phase G per core
        nxt = np.zeros((act.shape[0], dh), f32)
        twagg = dh + 3
        for c in range(ncores):
            pc = per_core[c]
            meta_nblk = meta["nblk"]
            for b in range(meta_nblk):
                nch = meta["ncha"][b] + meta["nchb"][b]
                o = meta["idx_off"][b]
                psum = np.zeros((128, twagg), f32)
                for j in range(nch):
                    sl = slice(o + j * 128, o + (j + 1) * 128)
                    rows = tab[pc["srcs"][sl]].astype(f32)   # gathered [128, tw]
                    asr = rows[:, dh] + rows[:, dh + 1]
                    base, w = meta["win"][b][j]
                    ad = np.zeros(128, f32)
                    v = pc["valid"][sl]
                    ad[v] = adst[c * nv + b * 128 + pc["dloc"][sl][v]]
                    # adst via bf16 hi/lo (psum-accumulated)
                    ahi = bf(ad).astype(f32)
                    alo = bf(ad - ahi).astype(f32)
                    ad = ahi + alo
                    e = asr + ad
                    t = np.where(e > 0, e, NEG_SLOPE * e)
                    p = np.exp(t - CEXP).astype(f32)
                    S = np.zeros((128, w), f32)
                    ee = np.arange(128)[v]
                    S[ee, pc["dloc"][sl][v] - base] = bf(p[v]).astype(f32)
                    psum[base:base + w] += S.T @ bf(rows[:, :twagg]).astype(f32)
                denom = np.maximum(psum[:, twagg - 1], 1e-30)
                res = psum[:, :dh] / denom[:, None]
                bw = meta["blkw"][b]
                nxt[c * nv + b * 128: c * nv + b * 128 + bw] = res[:bw]
        if li < 2:
            act = np.maximum(nxt, 0.0)
        else:
            outs = nxt[:, :N_CLS]
    return outs


# ----------------------------------------------------------------------------
# bass program
# ----------------------------------------------------------------------------

def build_nc(meta, ncores=NCORES, nv=NV, nlayers=3):
    import concourse.bass as bass
    import concourse.bacc as bacc
    import concourse.mybir as mybir
    import concourse.tile as tile

    f32, bf16, i16 = mybir.dt.float32, mybir.dt.bfloat16, mybir.dt.int16
    AF = mybir.ActivationFunctionType
    ALU = mybir.AluOpType

    nblk = meta["nblk"]
    blkw = meta["blkw"]
    ncha, nchb, win = meta["ncha"], meta["nchb"], meta["win"]
    nvp = nblk * 128                    # partition-padded slice rows
    NCHMAX = max(ncha[b] + nchb[b] for b in range(nblk))

    nc = bacc.Bacc("TRN2", target_bir_lowering=False, debug=False,
                   num_devices=ncores)

    xsl = nc.dram_tensor("xslice", [nvp, D_IN], f32, kind="ExternalInput")
    idxd = nc.dram_tensor("idx", [128, meta["tot_slots"] // 16], i16,
                          kind="ExternalInput")
    mbd = nc.dram_tensor("mblob", [128, meta["tot_mcols"]], bf16,
                         kind="ExternalInput")
    wa1 = nc.dram_tensor("wa1", [D_IN, D_H + 2], f32, kind="ExternalInput")
    wa2 = nc.dram_tensor("wa2", [D_H, D_H + 2], f32, kind="ExternalInput")
    wa3 = nc.dram_tensor("wa3", [D_H, N_CLS + 2], f32, kind="ExternalInput")
    identd = nc.dram_tensor("ident", [128, 128], f32, kind="ExternalInput")
    outd = nc.dram_tensor("out", [nvp, N_CLS], f32, kind="ExternalOutput")

    layers = [
        dict(dh=D_H, tw=TW12, wad=wa1, wcols=D_H + 2, relu=True),
        dict(dh=D_H, tw=TW12, wad=wa2, wcols=D_H + 2, relu=True),
        dict(dh=N_CLS, tw=TW3, wad=wa3, wcols=N_CLS + 2, relu=False),
    ][:nlayers]

    with tile.TileContext(nc) as tc:
        with (
            tc.tile_pool(name="const", bufs=1) as cpool,
            tc.tile_pool(name="xb", bufs=1) as xpool,
            tc.tile_pool(name="gath", bufs=2) as gpool,
            tc.tile_pool(name="mb", bufs=2) as mpool,
            tc.tile_pool(name="sc", bufs=3) as scpool,
            tc.tile_pool(name="sbld", bufs=4) as spool,
            tc.tile_pool(name="pack", bufs=2) as kpool,
            tc.tile_pool(name="eps", bufs=3) as epool,
            tc.tile_pool(name="pm", bufs=2, space="PSUM") as pm,
            tc.tile_pool(name="pa", bufs=2, space="PSUM") as pa,
            tc.tile_pool(name="pt", bufs=2, space="PSUM") as pt,
            tc.tile_pool(name="ph", bufs=2, space="PSUM") as ph,
            tc.tile_pool(name="dram", bufs=1, space="DRAM") as dpool,
        ):
            # ---- resident constants ----
            ident = cpool.tile([128, 128], f32, tag="ident")
            nc.sync.dma_start(ident[:], identd[:])
            idx_sb = cpool.tile([128, meta["tot_slots"] // 16], i16, tag="idx")
            nc.sync.dma_start(idx_sb[:], idxd[:])
            zeros128 = cpool.tile([128, 128], bf16, tag="z128")
            nc.vector.memset(zeros128[:], 0.0)
            negc = cpool.tile([128, 1], f32, tag="negc")
            nc.vector.memset(negc[:], -CEXP)
            w_sb = []
            for li, L in enumerate(layers):
                w = cpool.tile([128, 2, L["wcols"]], f32, tag=f"w{li}",
                               name=f"w_sb{li}")
                nc.sync.dma_start(
                    w[:], L["wad"].rearrange("(k p) n -> p k n", p=128)[:])
                w_sb.append(w)
            adst_hi = [cpool.tile([128, nblk], bf16, tag=f"ahi{li}",
                                  name=f"adst_hi{li}") for li in range(3)]
            adst_lo = [cpool.tile([128, nblk], bf16, tag=f"alo{li}",
                                  name=f"adst_lo{li}") for li in range(3)]

            # ---- activations (ping-pong, partition-major [128, nblk, D]) ----
            x_a = xpool.tile([128, nblk, D_IN], f32, tag="xa")
            x_b = xpool.tile([128, nblk, D_H], f32, tag="xb")
            nc.sync.dma_start(
                x_a[:], xsl.rearrange("(b p) f -> p b f", p=128)[:])
            out_sb = xpool.tile([128, nblk, N_CLS], f32, tag="osb")

            # ---- DRAM tables ----
            slices, tables = [], []
            for li, L in enumerate(layers):
                slices.append(dpool.tile([nv, L["tw"]], bf16,
                                         name=f"slice{li}"))
                tables.append(dpool.tile([ncores * nv, L["tw"]], bf16,
                                         addr_space="Shared",
                                         name=f"table{li}"))

            x_cur = x_a
            for li, L in enumerate(layers):
                dh, tw, wcols = L["dh"], L["tw"], L["wcols"]
                twagg = dh + 3
                # ================= Phase M =================
                for b in range(nblk):
                    xt_ps = pt.tile([128, 256], f32, tag="pt", padded_shape=[128, 512])
                    for k in range(2):
                        nc.tensor.transpose(
                            xt_ps[:, k * 128:(k + 1) * 128],
                            x_cur[:, b, k * 128:(k + 1) * 128], ident[:])
                    xt_sb = kpool.tile([128, 256], f32, tag="xt")
                    nc.vector.tensor_copy(xt_sb[:], xt_ps[:])
                    h_ps = ph.tile([128, wcols], f32, tag="ph", padded_shape=[128, 512])
                    for k in range(2):
                        nc.tensor.matmul(
                            h_ps[:], xt_sb[:, k * 128:(k + 1) * 128],
                            w_sb[li][:, k, :], start=(k == 0), stop=(k == 1))
                    packed = kpool.tile([128, tw], bf16, tag="pk")
                    nc.scalar.activation(packed[:, 0:dh], h_ps[:, 0:dh], AF.Copy)
                    nc.scalar.activation(packed[:, dh:dh + 1],
                                         h_ps[:, dh:dh + 1], AF.Copy)
                    hi32 = epool.tile([128, 1], f32, tag="hi32")
                    nc.vector.tensor_copy(hi32[:], packed[:, dh:dh + 1])
                    nc.vector.tensor_tensor(
                        out=packed[:, dh + 1:dh + 2], in0=h_ps[:, dh:dh + 1],
                        in1=hi32[:], op=ALU.subtract)
                    nc.vector.memset(packed[:, dh + 2:dh + 3], 1.0)
                    nc.vector.memset(packed[:, dh + 3:tw], 0.0)
                    # adst hi/lo -> resident columns
                    nc.scalar.activation(adst_hi[li][:, b:b + 1],
                                         h_ps[:, dh + 1:dh + 2], AF.Copy)
                    ahi32 = epool.tile([128, 1], f32, tag="ahi32")
                    nc.vector.tensor_copy(ahi32[:], adst_hi[li][:, b:b + 1])
                    nc.vector.tensor_tensor(
                        out=adst_lo[li][:, b:b + 1], in0=h_ps[:, dh + 1:dh + 2],
                        in1=ahi32[:], op=ALU.subtract)
                    nc.sync.dma_start(
                        slices[li][b * 128:b * 128 + blkw[b], :],
                        packed[0:blkw[b], :])
                # ================= AllGather =================
                nc.gpsimd.collective_compute(
                    "AllGather", mybir.AluOpType.bypass,
                    replica_groups=[list(range(ncores))],
                    ins=[slices[li].opt()], outs=[tables[li].opt()])
                # ================= Phase G =================
                if li < 2:
                    x_nxt = x_b if li == 0 else x_a
                table = tables[li]
                for b in range(nblk):
                    nA, nB = ncha[b], nchb[b]
                    nch = nA + nB
                    o16 = meta["idx_off"][b] // 16
                    gbA = gbB = None
                    if nA:
                        gbA = gpool.tile([128, nA, tw], bf16, tag="gA")
                        nc.gpsimd.dma_gather(
                            gbA[:], table[:, :],
                            idx_sb[:, o16:o16 + nA * 8], nA * 128, nA * 128, tw,
                            single_packet=False)
                    if nB:
                        gbB = gpool.tile([128, nB, tw], bf16, tag="gB")
                        o2 = o16 + nA * 8
                        nc.gpsimd.dma_gather(
                            gbB[:], table[SPLIT:, :],
                            idx_sb[:, o2:o2 + nB * 8], nB * 128, nB * 128, tw,
                            single_packet=False)
                    mb_sb = mpool.tile([128, meta["mblob_len"][b]], bf16,
                                       tag="mb")
                    mo = meta["mblob_off"][b]
                    nc.sync.dma_start(
                        mb_sb[:], mbd[:, mo:mo + meta["mblob_len"][b]])
                    # adst broadcast (tiny M01^T matmuls, hi+lo accumulate)
                    ps_a = pa.tile([128, nch], f32, tag="pa", padded_shape=[128, 512])
                    for j in range(nch):
                        base, w = win[b][j]
                        pr, tc0 = meta["m01t_pos"][b][j]
                        lhs = mb_sb[pr:pr + w, tc0:tc0 + 128]
                        nc.tensor.matmul(
                            ps_a[:, j:j + 1], lhs,
                            adst_hi[li][base:base + w, b:b + 1],
                            start=True, stop=False)
                        nc.tensor.matmul(
                            ps_a[:, j:j + 1], lhs,
                            adst_lo[li][base:base + w, b:b + 1],
                            start=False, stop=True)
                    # scalar path
                    epre = scpool.tile([128, nch], f32, tag="epre")
                    if nA:
                        nc.vector.tensor_tensor(
                            out=epre[:, 0:nA], in0=gbA[:, :, dh],
                            in1=gbA[:, :, dh + 1], op=ALU.add)
                    if nB:
                        nc.vector.tensor_tensor(
                            out=epre[:, nA:nch], in0=gbB[:, :, dh],
                            in1=gbB[:, :, dh + 1], op=ALU.add)
                    nc.vector.tensor_tensor(out=epre[:], in0=epre[:],
                                            in1=ps_a[:, 0:nch], op=ALU.add)
                    tt = scpool.tile([128, nch], f32, tag="tt")
                    nc.vector.tensor_scalar_mul(tt[:], epre[:], NEG_SLOPE)
                    nc.vector.tensor_tensor(out=tt[:], in0=tt[:], in1=epre[:],
                                            op=ALU.max)
                    pp = scpool.tile([128, nch], f32, tag="pp")
                    nc.scalar.activation(pp[:], tt[:], AF.Exp, bias=negc[:])
                    # main aggregation
                    ps_m = pm.tile([128, twagg], f32, tag="pm", padded_shape=[128, 512])
                    first = gbA if nA else gbB
                    nc.tensor.matmul(ps_m[:], zeros128[:],
                                     first[:, 0, 0:twagg],
                                     start=True, stop=False,
                                     skip_group_check=True)
                    for j in range(nch):
                        base, w = win[b][j]
                        gb, jj = (gbA, j) if j < nA else (gbB, j - nA)
                        S = spool.tile([128, w], bf16, tag="S")
                        col = meta["mblob_off"][b] + meta["m01_off"][b][j] - mo
                        nc.vector.tensor_scalar_mul(
                            S[:], mb_sb[:, col:col + w], pp[:, j:j + 1])
                        nc.tensor.matmul(
                            ps_m[base:base + w, 0:twagg], S[:],
                            gb[:, jj, 0:twagg], start=False,
                            stop=(j == nch - 1), skip_group_check=True)
                    # normalize (+relu) epilogue
                    dn = epool.tile([128, 1], f32, tag="dn")
                    nc.vector.tensor_scalar_max(
                        dn[:], ps_m[:, twagg - 1:twagg], 1e-30)
                    rc = epool.tile([128, 1], f32, tag="rc")
                    nc.vector.reciprocal(rc[:], dn[:])
                    if li < 2:
                        nc.vector.tensor_scalar(
                            out=x_nxt[:, b, :], in0=ps_m[:, 0:dh],
                            scalar1=rc[:], scalar2=0.0,
                            op0=ALU.mult, op1=ALU.max)
                    else:
                        nc.vector.tensor_scalar_mul(
                            out_sb[:, b, :], ps_m[:, 0:dh], rc[:])
                if li < 2:
                    x_cur = x_nxt
            if nlayers == 3:
                nc.sync.dma_start(
                    outd.rearrange("(b p) f -> p b f", p=128)[:], out_sb[:])
            else:
                # debug: dump first N_CLS cols of the last activation
                nc.sync.dma_start(
                    outd.rearrange("(b p) f -> p b f", p=128)[:],
                    x_cur[:, :, 0:N_CLS])
    nc.compile()
    return nc


# ----------------------------------------------------------------------------
# entry point
# ----------------------------------------------------------------------------

_CACHE = {}


def _prep_all(x, edge_index, params):
    meta, per_core = preprocess(np.asarray(edge_index))
    waugs = [make_waug(params["W1"], params["a_src1"], params["a_dst1"]),
             make_waug(params["W2"], params["a_src2"], params["a_dst2"]),
             make_waug(params["W3"], params["a_src3"], params["a_dst3"])]
    x = np.asarray(x, np.float32)
    nvp = meta["nblk"] * 128
    in_maps = []
    for c in range(NCORES):
        xs = np.zeros((nvp, D_IN), np.float32)
        xs[:NV] = x[c * NV:(c + 1) * NV]
        in_maps.append({
            "xslice": xs,
            "idx": per_core[c]["idx"],
            "mblob": per_core[c]["mblob"],
            "wa1": waugs[0], "wa2": waugs[1], "wa3": waugs[2],
            "ident": np.eye(128, dtype=np.float32),
        })
    return meta, per_core, waugs, in_maps


def kernel(x, edge_index, W1, a_src1, a_dst1, b1, W2, a_src2, a_dst2, b2,
           W3, a_src3, a_dst3, b3):
    params = dict(W1=W1, a_src1=a_src1, a_dst1=a_dst1, W2=W2, a_src2=a_src2,
                  a_dst2=a_dst2, W3=W3, a_src3=a_src3, a_dst3=a_dst3)
    for b in (b1, b2, b3):
        assert np.abs(np.asarray(b)).max() == 0.0, "nonzero bias unsupported"
    meta, per_core, waugs, in_maps = _prep_all(x, edge_index, params)

    key = "nc"
    if key not in _CACHE:
        _CACHE[key] = build_nc(meta)
    nc = _CACHE[key]

    from concourse.bass_utils import run_bass_kernel_spmd
    res = run_bass_kernel_spmd(nc, in_maps, list(range(NCORES)))
    out = np.concatenate([res.results[c]["out"][:NV] for c in range(NCORES)],
                         axis=0)
    return out.astype(np.float32)
